# revision 1
# baseline (speedup 1.0000x reference)
"""Trainium2 Bass kernel for ContrastiveNet loss.

Algorithm (per core k of 8, SPMD):
  - host: xt_rot = x.T rolled so core k's 512 anchor rows sit at columns 0..511
  - device: cast xT->bf16, norms via squares + ones-matmul colsum,
    gram G = Xblk @ X.T in bf16 (PE), sim = G * invw_row * invw_col (DVE, ->bf16),
    per-pair logit gather via gpsimd.local_scatter (per-partition scatter of each
    sim row into pair-slot layout, duplicate columns handled by chained levels),
    masked exp/logsumexp (ACT+DVE), per-core partial sum -> [1,1].
  - host: sum 8 partials / P.
"""
import os
import sys
import numpy as np
import ml_dtypes

try:
    import concourse  # noqa: F401
except ImportError:
    sys.path.insert(0, "/opt/trn_rl_repo")

from contextlib import ExitStack

import concourse.bass as bass
import concourse.tile as tile
from concourse import bacc, mybir
from concourse._compat import with_exitstack
from concourse.bass_utils import run_bass_kernel_spmd

BF16 = ml_dtypes.bfloat16
F32 = mybir.dt.float32
DBF = mybir.dt.bfloat16
I16 = mybir.dt.int16

B, D, J = 4096, 2048, 11
NCORES, RPC, NT, NKT = 8, 512, 4, 16  # rows/core, row-tiles/core, k-tiles
TEMP = 0.1
AF = mybir.ActivationFunctionType
ALU = mybir.AluOpType
AX = mybir.AxisListType


# ---------------------------------------------------------------- host prep
def build_plan(anchor_idx, pos_idx, neg_idx):
    """Scatter planes; plane0 column indices are per-core ROTATED by -512k."""
    r = anchor_idx.astype(np.int64)
    cols = np.concatenate([pos_idx[:, None], neg_idx], axis=1).astype(np.int64)
    P = r.shape[0]

    order = np.argsort(r, kind="stable")
    r_sorted = r[order]
    first = np.r_[True, r_sorted[1:] != r_sorted[:-1]]
    gid = np.cumsum(first) - 1
    rank_sorted = np.arange(P) - np.flatnonzero(first)[gid]
    srank = np.empty(P, np.int64)
    srank[order] = rank_sorted
    n_per_row = np.bincount(r, minlength=B)
    SP = int(max(n_per_row.max(), 1))
    NE = SP * J + (SP * J) % 2
    assert NE * 32 < 2**16

    er = np.repeat(r, J)
    ec = cols.ravel()
    eslot = np.repeat(srank, J) * J + np.tile(np.arange(J), P)
    key = er * B + ec
    o2 = np.argsort(key, kind="stable")
    k_sorted = key[o2]
    first2 = np.r_[True, k_sorted[1:] != k_sorted[:-1]]
    gid2 = np.cumsum(first2) - 1
    occ_sorted = np.arange(P * J) - np.flatnonzero(first2)[gid2]
    occ = np.empty(P * J, np.int64)
    occ[o2] = occ_sorted
    L = int(occ.max())

    eslot_sorted = eslot[o2]
    prev_slot_sorted = np.empty(P * J, np.int64)
    prev_slot_sorted[0] = -1
    prev_slot_sorted[1:] = eslot_sorted[:-1]
    prev_slot = np.empty(P * J, np.int64)
    prev_slot[o2] = prev_slot_sorted

    core = er // RPC
    t = (er % RPC) // 128
    pp = er % 128
    ec_rot = (ec - core * RPC) % B  # per-core rotated column index

    plane0 = np.full((NCORES, NT, 128, B), -1, np.int16)
    m0 = occ == 0
    plane0[core[m0], t[m0], pp[m0], ec_rot[m0]] = eslot[m0].astype(np.int16)

    planes = []
    for q in range(1, L + 1):
        pl = np.full((NCORES, NT, 128, NE), -1, np.int16)
        mq = occ == q
        pl[core[mq], t[mq], pp[mq], prev_slot[mq]] = eslot[mq].astype(np.int16)
        planes.append(pl)

    nmat = n_per_row.reshape(NCORES, NT, 128)
    maskplane = ((np.arange(NE)[None, None, None, :] // J) < nmat[..., None]).astype(BF16)
    pairmask = (np.arange(SP)[None, None, None, :] < nmat[..., None]).astype(BF16)
    return dict(plane0=plane0, planes=planes, maskplane=maskplane,
                pairmask=pairmask, SP=SP, NE=NE, L=L)


# ------------------------------------------------------------- device kernel
@with_exitstack
def _build(ctx: ExitStack, tc: "tile.TileContext", io: dict, SP: int, NE: int, L: int):
    nc = tc.nc
    xt, pl0_d, mask_d, pm_d, out_d = io["xt"], io["plane0"], io["mask"], io["pm"], io["out"]
    plq_d = [io[f"plane{q}"] for q in range(1, L + 1)]

    consts = ctx.enter_context(tc.tile_pool(name="consts", bufs=1))
    ones_bf = consts.tile([128, 1], DBF, tag="ones_bf")
    nc.vector.memset(ones_bf[:], 1.0)
    ones_f32r = consts.tile([1, 128], F32, tag="ones_f32r")
    nc.vector.memset(ones_f32r[:], 1.0)
    ones_f32c = consts.tile([128, 1], F32, tag="ones_f32c")
    nc.vector.memset(ones_f32c[:], 1.0)
    neg30 = consts.tile([128, 1], F32, tag="neg30")
    nc.vector.memset(neg30[:], -30.0)

    ypool = ctx.enter_context(tc.tile_pool(name="y", bufs=1))
    y = [ypool.tile([128, B], DBF, tag=f"y{kt}", name=f"y{kt}") for kt in range(NKT)]

    npool = ctx.enter_context(tc.tile_pool(name="norms", bufs=1))
    invw = npool.tile([1, B], F32, tag="invw")
    invw_bc = npool.tile([128, B], DBF, tag="invw_bc")
    invwT = npool.tile([128, NT], F32, tag="invwT")

    # ---- phase 1: load, cast, squares, colsum
    with tc.tile_pool(name="p1psum", bufs=1, space="PSUM") as p1psum, \
         tc.tile_pool(name="stage", bufs=2) as stpool, \
         tc.tile_pool(name="sq", bufs=2) as sqpool:
        norm_ps = p1psum.tile([1, B], F32, tag="norm_ps")
        for kt in range(NKT):
            st = stpool.tile([128, B], F32, tag="stage")
            nc.sync.dma_start(st[:], xt[kt * 128:(kt + 1) * 128, :])
            sq = sqpool.tile([128, B], DBF, tag="sq")
            if kt % 2 == 0:
                nc.scalar.copy(y[kt][:], st[:])
                nc.vector.tensor_tensor(sq[:], st[:], st[:], ALU.mult)
            else:
                nc.vector.tensor_copy(y[kt][:], st[:])
                nc.scalar.activation(sq[:], st[:], AF.Square)
            for nch in range(8):
                nc.tensor.matmul(
                    norm_ps[:, nch * 512:(nch + 1) * 512],
                    lhsT=ones_bf[:, 0:1], rhs=sq[:, nch * 512:(nch + 1) * 512],
                    start=(kt == 0), stop=(kt == NKT - 1),
                )
        nc.scalar.copy(invw[:], norm_ps[:])

    # ---- phase 2: invw = sqrt(10/norm2) = sqrt(10)/||x|| (in-place on invw)
    nc.vector.reciprocal(invw[:], invw[:])
    nc.scalar.activation(invw[:], invw[:], AF.Sqrt, scale=1.0 / TEMP)
    with tc.tile_pool(name="p2psum", bufs=1, space="PSUM") as p2psum:
        psT = p2psum.tile([128, NT], F32, tag="psT")
        for mt in range(NT):
            nc.tensor.matmul(
                psT[:, mt:mt + 1],
                lhsT=invw[0:1, mt * 128:(mt + 1) * 128], rhs=ones_f32r[0:1, 0:1],
                start=True, stop=True,
            )
        nc.scalar.copy(invwT[:], psT[:])
        for nch in range(8):
            bc = p2psum.tile([128, 512], F32, tag="bc")
            nc.tensor.matmul(
                bc[:],
                lhsT=ones_f32r[0:1, :], rhs=invw[0:1, nch * 512:(nch + 1) * 512],
                start=True, stop=True,
            )
            nc.scalar.copy(invw_bc[:, nch * 512:(nch + 1) * 512], bc[:])

    # ---- phases 3+4: gram per (mt, half), fused normalize; scatter + loss per mt
    lpool = ctx.enter_context(tc.tile_pool(name="loss", bufs=1))
    acc4 = lpool.tile([128, NT], F32, tag="acc4")

    with tc.tile_pool(name="gpsum", bufs=2, space="PSUM") as gpsum, \
         tc.tile_pool(name="gbf", bufs=2) as gbfpool, \
         tc.tile_pool(name="pl", bufs=1) as plpool, \
         tc.tile_pool(name="slots", bufs=2) as slpool, \
         tc.tile_pool(name="elb", bufs=1) as elpool:
        for mt in range(NT):
            gbf = gbfpool.tile([128, B], DBF, tag="gbf")
            for half in range(2):
                gps = gpsum.tile([128, 2048], F32, tag="gram")
                for kt in range(NKT):
                    for nch in range(4):
                        nc.tensor.matmul(
                            gps[:, nch * 512:(nch + 1) * 512],
                            lhsT=y[kt][:, mt * 128:(mt + 1) * 128],
                            rhs=y[kt][:, half * 2048 + nch * 512: half * 2048 + (nch + 1) * 512],
                            start=(kt == 0), stop=(kt == NKT - 1),
                        )
                nc.vector.scalar_tensor_tensor(
                    gbf[:, half * 2048:(half + 1) * 2048], gps[:],
                    invwT[:, mt:mt + 1],
                    invw_bc[:, half * 2048:(half + 1) * 2048],
                    ALU.mult, ALU.mult,
                )

            # scatter chain
            pl0 = plpool.tile([128, B], I16, tag="pl0")
            nc.sync.dma_start(pl0[:], pl0_d[mt])
            s_lv = slpool.tile([128, NE], DBF, tag=f"slv0")
            nc.gpsimd.local_scatter(s_lv[:], gbf[:], pl0[:], 128, NE, B)
            s_all = slpool.tile([128, NE], DBF, tag="s_all")
            nc.vector.tensor_copy(s_all[:], s_lv[:])
            for q in range(1, L + 1):
                plq = plpool.tile([128, NE], I16, tag=f"plq{q}")
                nc.sync.dma_start(plq[:], plq_d[q - 1][mt])
                s_nx = slpool.tile([128, NE], DBF, tag=f"slv{q % 2 + 1}")
                nc.gpsimd.local_scatter(s_nx[:], s_lv[:], plq[:], 128, NE, NE)
                nc.vector.tensor_tensor(s_all[:], s_all[:], s_nx[:], ALU.add)
                s_lv = s_nx

            # masked exp / logsumexp / accumulate
            msk = elpool.tile([128, NE], DBF, tag="msk")
            nc.sync.dma_start(msk[:], mask_d[mt])
            pm = elpool.tile([128, SP], DBF, tag="pm")
            nc.sync.dma_start(pm[:], pm_d[mt])
            arg = elpool.tile([128, NE], F32, tag="arg")
            nc.vector.scalar_tensor_tensor(arg[:], s_all[:], 30.0, msk[:], ALU.add, ALU.mult)
            ebuf = elpool.tile([128, NE], F32, tag="ebuf")
            nc.scalar.activation(ebuf[:], arg[:], AF.Exp, bias=neg30[:, 0:1])
            denom = elpool.tile([128, SP], F32, tag="denom")
            e3 = ebuf[:, 0:SP * J].rearrange("p (s j) -> p s j", j=J)
            nc.vector.tensor_reduce(denom[:], e3, AX.X, ALU.add)
            lnd = elpool.tile([128, SP], F32, tag="lnd")
            nc.scalar.activation(lnd[:], denom[:], AF.Ln)
            diff = elpool.tile([128, SP], F32, tag="diff")
            l0 = s_all[:, 0:SP * J].rearrange("p (s j) -> p s j", j=J)[:, :, 0]
            nc.vector.scalar_tensor_tensor(diff[:], l0, -1.0, lnd[:], ALU.mult, ALU.add)
            scrap = elpool.tile([128, SP], F32, tag="scrap")
            nc.vector.scalar_tensor_tensor(
                scrap[:], diff[:], 1.0, pm[:], ALU.mult, ALU.mult,
                accum_out=acc4[:, mt:mt + 1],
            )

    # ---- phase 5: total
    with tc.tile_pool(name="p5psum", bufs=1, space="PSUM") as p5psum:
        tot = lpool.tile([128, 1], F32, tag="tot")
        nc.vector.tensor_reduce(tot[:], acc4[:], AX.X, ALU.add)
        ps = p5psum.tile([1, 1], F32, tag="ps_out")
        nc.tensor.matmul(ps[:], lhsT=tot[:], rhs=ones_f32c[:, 0:1],
                         start=True, stop=True)
        res = lpool.tile([1, 1], F32, tag="res")
        nc.scalar.copy(res[:], ps[:])
        nc.sync.dma_start(out_d[:], res[:])


def build_nc(SP, NE, L, enable_asserts=False):
    nc = bacc.Bacc("TRN2", target_bir_lowering=False, debug=False,
                   enable_asserts=enable_asserts, num_devices=NCORES)
    io = {
        "xt": nc.dram_tensor("xt", [D, B], F32, kind="ExternalInput").ap(),
        "plane0": nc.dram_tensor("plane0", [NT, 128, B], I16, kind="ExternalInput").ap(),
        "mask": nc.dram_tensor("mask", [NT, 128, NE], DBF, kind="ExternalInput").ap(),
        "pm": nc.dram_tensor("pm", [NT, 128, SP], DBF, kind="ExternalInput").ap(),
        "out": nc.dram_tensor("out", [1, 1], F32, kind="ExternalOutput").ap(),
    }
    for q in range(1, L + 1):
        io[f"plane{q}"] = nc.dram_tensor(
            f"plane{q}", [NT, 128, NE], I16, kind="ExternalInput").ap()
    with tile.TileContext(nc) as tc:
        _build(tc, io, SP, NE, L)
    nc.compile()
    return nc


def make_in_maps(x, plan):
    xT = np.ascontiguousarray(np.asarray(x, np.float32).T)
    in_maps = []
    for k in range(NCORES):
        m = {
            "xt": np.ascontiguousarray(np.roll(xT, -RPC * k, axis=1)),
            "plane0": plan["plane0"][k],
            "mask": plan["maskplane"][k],
            "pm": plan["pairmask"][k],
        }
        for q in range(1, plan["L"] + 1):
            m[f"plane{q}"] = plan["planes"][q - 1][k]
        in_maps.append(m)
    return in_maps


def kernel(**inputs):
    x = np.asarray(inputs["x"], np.float32)
    anchor_idx = np.asarray(inputs["anchor_idx"])
    pos_idx = np.asarray(inputs["pos_idx"])
    neg_idx = np.asarray(inputs["neg_idx"])
    P = anchor_idx.shape[0]

    plan = build_plan(anchor_idx, pos_idx, neg_idx)
    nc = build_nc(plan["SP"], plan["NE"], plan["L"])
    in_maps = make_in_maps(x, plan)
    res = run_bass_kernel_spmd(nc, in_maps, list(range(NCORES)))
    total = sum(float(res.results[k]["out"][0, 0]) for k in range(NCORES))
    return np.float32(total / P)



# revision 5
# speedup vs baseline: 2.6957x; 2.6957x over previous
"""Trainium2 Bass kernel for ContrastiveNet loss (v2: fp8 DoubleRow).

Algorithm (per core k of 8, SPMD):
  - host: x cast to fp8e4 (e4m3, +-240), rolled so core k's 512 anchor rows sit
    at columns 0..511, laid out [128, 16, B] for DoubleRow k-pairing.
  - device:
      load y (fp8), squares -> sq (fp8, ACT/DVE split) during load,
      colsum-of-squares via DoubleRow matmul vs ones -> norm^2 replicated
      across partitions in PSUM, Rsqrt(0.1*n2) -> invw_bc [128,B] bf16,
      row scales invwT[p,mt] = diag pick of invw_bc via identity mask,
      gram G = Xblk @ X.T in fp8 DoubleRow (PE, 0.5 cyc/col),
      sim = G * invwT * invw_bc (DVE stt -> bf16),
      per-pair logit gather via gpsimd.local_scatter chains,
      exp/logsumexp (no slot mask needed: invalid pairs zeroed by pairmask),
      per-core partial sum -> [1,1].
  - host: sum 8 partials / P.
"""
import os
import sys
import numpy as np
import ml_dtypes

try:
    import concourse  # noqa: F401
except ImportError:
    sys.path.insert(0, "/opt/trn_rl_repo")

from contextlib import ExitStack

import concourse.bass as bass
import concourse.tile as tile
from concourse import bacc, mybir
from concourse._compat import with_exitstack
from concourse.bass_utils import run_bass_kernel_spmd

BF16 = ml_dtypes.bfloat16
FP8 = ml_dtypes.float8_e4m3
F32 = mybir.dt.float32
DBF = mybir.dt.bfloat16
F8 = mybir.dt.float8e4
I16 = mybir.dt.int16

B, D, J = 4096, 2048, 11
NCORES, RPC, NT, NKT, NKP = 8, 512, 4, 16, 8
TEMP = 0.1
AF = mybir.ActivationFunctionType
ALU = mybir.AluOpType
AX = mybir.AxisListType
DR = mybir.MatmulPerfMode.DoubleRow


# ---------------------------------------------------------------- host prep
def build_plan(anchor_idx, pos_idx, neg_idx):
    """Scatter planes; plane0 column indices are per-core ROTATED by -512k."""
    r = anchor_idx.astype(np.int64)
    cols = np.concatenate([pos_idx[:, None], neg_idx], axis=1).astype(np.int64)
    P = r.shape[0]

    order = np.argsort(r, kind="stable")
    r_sorted = r[order]
    first = np.r_[True, r_sorted[1:] != r_sorted[:-1]]
    gid = np.cumsum(first) - 1
    rank_sorted = np.arange(P) - np.flatnonzero(first)[gid]
    srank = np.empty(P, np.int64)
    srank[order] = rank_sorted
    n_per_row = np.bincount(r, minlength=B)
    SP = int(max(n_per_row.max(), 1))
    NE = SP * J + (SP * J) % 2
    assert NE * 32 < 2**16

    er = np.repeat(r, J)
    ec = cols.ravel()
    eslot = np.repeat(srank, J) * J + np.tile(np.arange(J), P)
    key = er * B + ec
    o2 = np.argsort(key, kind="stable")
    k_sorted = key[o2]
    first2 = np.r_[True, k_sorted[1:] != k_sorted[:-1]]
    gid2 = np.cumsum(first2) - 1
    occ_sorted = np.arange(P * J) - np.flatnonzero(first2)[gid2]
    occ = np.empty(P * J, np.int64)
    occ[o2] = occ_sorted
    L = int(occ.max())

    eslot_sorted = eslot[o2]
    prev_slot_sorted = np.empty(P * J, np.int64)
    prev_slot_sorted[0] = -1
    prev_slot_sorted[1:] = eslot_sorted[:-1]
    prev_slot = np.empty(P * J, np.int64)
    prev_slot[o2] = prev_slot_sorted

    core = er // RPC
    t = (er % RPC) // 128
    pp = er % 128
    ec_rot = (ec - core * RPC) % B  # per-core rotated column index

    plane0 = np.full((NCORES, NT, 128, B), -1, np.int16)
    m0 = occ == 0
    plane0[core[m0], t[m0], pp[m0], ec_rot[m0]] = eslot[m0].astype(np.int16)

    planes = []
    for q in range(1, L + 1):
        pl = np.full((NCORES, NT, 128, NE), -1, np.int16)
        mq = occ == q
        pl[core[mq], t[mq], pp[mq], prev_slot[mq]] = eslot[mq].astype(np.int16)
        planes.append(pl)

    nmat = n_per_row.reshape(NCORES, NT, 128)
    pairmask = (np.arange(SP)[None, None, None, :] < nmat[..., None]).astype(BF16)
    return dict(plane0=plane0, planes=planes, pairmask=pairmask, SP=SP, NE=NE, L=L)


# ------------------------------------------------------------- device kernel
@with_exitstack
def _build(ctx: ExitStack, tc: "tile.TileContext", io: dict, SP: int, NE: int, L: int):
    nc = tc.nc
    y_d, ident_d, pl0_d, pm_d, out_d = (
        io["y8"], io["ident"], io["plane0"], io["pm"], io["out"])
    plq_d = [io[f"plane{q}"] for q in range(1, L + 1)]

    consts = ctx.enter_context(tc.tile_pool(name="consts", bufs=1))
    ones8 = consts.tile([128, 2, 128], F8, tag="ones8")
    nc.vector.memset(ones8[:], 1.0)
    ones_f32c = consts.tile([128, 1], F32, tag="ones_f32c")
    nc.vector.memset(ones_f32c[:], 1.0)
    ident = consts.tile([128, 128], DBF, tag="ident")
    nc.scalar.dma_start(ident[:], ident_d[:])

    ypool = ctx.enter_context(tc.tile_pool(name="y", bufs=1))
    y = ypool.tile([128, NKT, B], F8, tag="y", name="y")

    npool = ctx.enter_context(tc.tile_pool(name="norms", bufs=1))
    invw_bc = npool.tile([128, B], DBF, tag="invw_bc")
    invwT = npool.tile([128, NT], F32, tag="invwT")
    scrapT = npool.tile([128, 128], DBF, tag="scrapT")

    lpool = ctx.enter_context(tc.tile_pool(name="loss", bufs=1))
    acc4 = lpool.tile([128, NT], F32, tag="acc4")

    # ---- x load (SP queue) + squares (ACT even-kt / DVE odd-kt) during load
    sqpool = ctx.enter_context(tc.tile_pool(name="sq", bufs=1))
    sq = sqpool.tile([128, NKT, B], F8, tag="sq", name="sq")
    for kp in range(NKP):
        nc.sync.dma_start(y[:, 2 * kp:2 * kp + 2, :], y_d[:, 2 * kp:2 * kp + 2, :])
        nc.scalar.activation(sq[:, 2 * kp:2 * kp + 1, :],
                             y[:, 2 * kp:2 * kp + 1, :], AF.Square)
        nc.vector.tensor_tensor(sq[:, 2 * kp + 1:2 * kp + 2, :],
                                y[:, 2 * kp + 1:2 * kp + 2, :],
                                y[:, 2 * kp + 1:2 * kp + 2, :], ALU.mult)

    with tc.tile_pool(name="gpsum", bufs=2, space="PSUM") as gpsum, \
         tc.tile_pool(name="gbf", bufs=2) as gbfpool, \
         tc.tile_pool(name="pl0", bufs=2) as pl0pool, \
         tc.tile_pool(name="plq", bufs=2) as plqpool, \
         tc.tile_pool(name="slots", bufs=2) as slpool, \
         tc.tile_pool(name="elb", bufs=1) as elpool:

        # ---- colsum of squares (DoubleRow vs fp8 ones), replicated 128 rows
        for half in range(2):
            cs = gpsum.tile([128, 2048], F32, tag="gram")
            for kp in range(NKP):
                for chk in range(4):
                    c0 = half * 2048 + chk * 512
                    nc.tensor.matmul(
                        cs[:, chk * 512:(chk + 1) * 512],
                        lhsT=ones8[:, 0:2, :],
                        rhs=sq[:, 2 * kp:2 * kp + 2, c0:c0 + 512],
                        start=(kp == 0), stop=(kp == NKP - 1),
                        perf_mode=DR,
                    )
            # invw_bc = sqrt((1/TEMP) / norm^2) = sqrt(10)/||x||
            rcp = npool.tile([128, 2048], F32, tag="rcp")
            nc.vector.reciprocal(rcp[:], cs[:])
            nc.scalar.activation(invw_bc[:, half * 2048:(half + 1) * 2048],
                                 rcp[:], AF.Sqrt, scale=1.0 / TEMP)

        # row scales: invwT[p, mt] = invw_bc[p, mt*128+p] (diag pick)
        for mt in range(NT):
            nc.vector.scalar_tensor_tensor(
                scrapT[:], invw_bc[:, mt * 128:(mt + 1) * 128], 1.0, ident[:],
                ALU.mult, ALU.mult, accum_out=invwT[:, mt:mt + 1])

        # prefetch mt0 planes on ACT queue (lands during colsum)
        pl0_t = {0: pl0pool.tile([128, B], I16, tag="pl0", name="pl0_0")}
        nc.scalar.dma_start(pl0_t[0][:], pl0_d[0])
        plq_t = {(0, q): plqpool.tile([128, NE], I16, tag=f"plq{q}", name=f"plq_0_{q}")
                 for q in range(1, L + 1)}
        for q in range(1, L + 1):
            nc.scalar.dma_start(plq_t[(0, q)][:], plq_d[q - 1][0])
        pm_t = {0: elpool.tile([128, SP], DBF, tag="pm0", name="pm_0")}
        nc.scalar.dma_start(pm_t[0][:], pm_d[0])

        # ---- per row-tile: gram (DoubleRow), normalize, scatter, loss
        for mt in range(NT):
            gbf = gbfpool.tile([128, B], DBF, tag="gbf")
            for half in range(2):
                gps = gpsum.tile([128, 2048], F32, tag="gram")
                for kp in range(NKP):
                    for chk in range(4):
                        c0 = half * 2048 + chk * 512
                        nc.tensor.matmul(
                            gps[:, chk * 512:(chk + 1) * 512],
                            lhsT=y[:, 2 * kp:2 * kp + 2, mt * 128:(mt + 1) * 128],
                            rhs=y[:, 2 * kp:2 * kp + 2, c0:c0 + 512],
                            start=(kp == 0), stop=(kp == NKP - 1),
                            perf_mode=DR,
                        )
                nc.vector.scalar_tensor_tensor(
                    gbf[:, half * 2048:(half + 1) * 2048], gps[:],
                    invwT[:, mt:mt + 1],
                    invw_bc[:, half * 2048:(half + 1) * 2048],
                    ALU.mult, ALU.mult,
                )

            # prefetch next tile's planes (ACT queue)
            if mt + 1 < NT:
                pl0_t[mt + 1] = pl0pool.tile([128, B], I16, tag="pl0", name=f"pl0_{mt+1}")
                nc.scalar.dma_start(pl0_t[mt + 1][:], pl0_d[mt + 1])
                for q in range(1, L + 1):
                    plq_t[(mt + 1, q)] = plqpool.tile([128, NE], I16, tag=f"plq{q}", name=f"plq_{mt+1}_{q}")
                    nc.scalar.dma_start(plq_t[(mt + 1, q)][:], plq_d[q - 1][mt + 1])
                pm_t[mt + 1] = elpool.tile([128, SP], DBF, tag=f"pm{(mt + 1) % 2}", name=f"pm_{mt+1}")
                nc.scalar.dma_start(pm_t[mt + 1][:], pm_d[mt + 1])

            # scatter chain
            s_lv = slpool.tile([128, NE], DBF, tag="slv0")
            nc.gpsimd.local_scatter(s_lv[:], gbf[:], pl0_t[mt][:], 128, NE, B)
            s_all = slpool.tile([128, NE], DBF, tag="s_all")
            nc.vector.tensor_copy(s_all[:], s_lv[:])
            for q in range(1, L + 1):
                s_nx = slpool.tile([128, NE], DBF, tag=f"slv{q % 2 + 1}")
                nc.gpsimd.local_scatter(s_nx[:], s_lv[:], plq_t[(mt, q)][:],
                                        128, NE, NE)
                nc.vector.tensor_tensor(s_all[:], s_all[:], s_nx[:], ALU.add)
                s_lv = s_nx

            # exp / logsumexp / accumulate (invalid pairs masked by pm only)
            ebuf = elpool.tile([128, NE], F32, tag="ebuf")
            nc.scalar.activation(ebuf[:], s_all[:], AF.Exp)
            denom = elpool.tile([128, SP], F32, tag="denom")
            e3 = ebuf[:, 0:SP * J].rearrange("p (s j) -> p s j", j=J)
            nc.vector.tensor_reduce(denom[:], e3, AX.X, ALU.add)
            lnd = elpool.tile([128, SP], F32, tag="lnd")
            nc.scalar.activation(lnd[:], denom[:], AF.Ln)
            diff = elpool.tile([128, SP], F32, tag="diff")
            l0 = s_all[:, 0:SP * J].rearrange("p (s j) -> p s j", j=J)[:, :, 0]
            nc.vector.scalar_tensor_tensor(diff[:], l0, -1.0, lnd[:],
                                           ALU.mult, ALU.add)
            scrap = elpool.tile([128, SP], F32, tag="scrap")
            nc.vector.scalar_tensor_tensor(
                scrap[:], diff[:], 1.0, pm_t[mt][:], ALU.mult, ALU.mult,
                accum_out=acc4[:, mt:mt + 1],
            )

    # ---- total
    with tc.tile_pool(name="p5psum", bufs=1, space="PSUM") as p5psum:
        tot = lpool.tile([128, 1], F32, tag="tot")
        nc.vector.tensor_reduce(tot[:], acc4[:], AX.X, ALU.add)
        ps = p5psum.tile([1, 1], F32, tag="ps_out")
        nc.tensor.matmul(ps[:], lhsT=tot[:], rhs=ones_f32c[:, 0:1],
                         start=True, stop=True)
        res = lpool.tile([1, 1], F32, tag="res")
        nc.scalar.copy(res[:], ps[:])
        nc.sync.dma_start(out_d[:], res[:])


def build_nc(SP, NE, L, enable_asserts=False):
    nc = bacc.Bacc("TRN2", target_bir_lowering=False, debug=False,
                   enable_asserts=enable_asserts, num_devices=NCORES)
    io = {
        "y8": nc.dram_tensor("y8", [128, NKT, B], F8, kind="ExternalInput").ap(),
        "ident": nc.dram_tensor("ident", [128, 128], DBF, kind="ExternalInput").ap(),
        "plane0": nc.dram_tensor("plane0", [NT, 128, B], I16, kind="ExternalInput").ap(),
        "pm": nc.dram_tensor("pm", [NT, 128, SP], DBF, kind="ExternalInput").ap(),
        "out": nc.dram_tensor("out", [1, 1], F32, kind="ExternalOutput").ap(),
    }
    for q in range(1, L + 1):
        io[f"plane{q}"] = nc.dram_tensor(
            f"plane{q}", [NT, 128, NE], I16, kind="ExternalInput").ap()
    with tile.TileContext(nc) as tc:
        _build(tc, io, SP, NE, L)
    nc.compile()
    return nc


def make_in_maps(x, plan):
    x8 = np.clip(np.asarray(x, np.float32), -240.0, 240.0).astype(FP8)
    ident = np.eye(128, dtype=BF16)
    in_maps = []
    for k in range(NCORES):
        xr = np.roll(x8, -RPC * k, axis=0)          # [B, D]
        y8 = np.ascontiguousarray(
            xr.T.reshape(NKT, 128, B).transpose(1, 0, 2))  # [128, NKT, B]
        m = {
            "y8": y8,
            "ident": ident,
            "plane0": plan["plane0"][k],
            "pm": plan["pairmask"][k],
        }
        for q in range(1, plan["L"] + 1):
            m[f"plane{q}"] = plan["planes"][q - 1][k]
        in_maps.append(m)
    return in_maps


def kernel(**inputs):
    x = np.asarray(inputs["x"], np.float32)
    anchor_idx = np.asarray(inputs["anchor_idx"])
    pos_idx = np.asarray(inputs["pos_idx"])
    neg_idx = np.asarray(inputs["neg_idx"])
    P = anchor_idx.shape[0]

    plan = build_plan(anchor_idx, pos_idx, neg_idx)
    nc = build_nc(plan["SP"], plan["NE"], plan["L"])
    in_maps = make_in_maps(x, plan)
    res = run_bass_kernel_spmd(nc, in_maps, list(range(NCORES)))
    total = sum(float(res.results[k]["out"][0, 0]) for k in range(NCORES))
    return np.float32(total / P)


# revision 8
# speedup vs baseline: 2.7189x; 1.0086x over previous
"""Trainium2 Bass kernel for ContrastiveNet loss (v2: fp8 DoubleRow).

Algorithm (per core k of 8, SPMD):
  - host: x cast to fp8e4 (e4m3, +-240), rolled so core k's 512 anchor rows sit
    at columns 0..511, laid out [128, 16, B] for DoubleRow k-pairing.
  - device:
      load y (fp8), squares -> sq (fp8, ACT/DVE split) during load,
      colsum-of-squares via DoubleRow matmul vs ones -> norm^2 replicated
      across partitions in PSUM, Rsqrt(0.1*n2) -> invw_bc [128,B] bf16,
      row scales invwT[p,mt] = diag pick of invw_bc via identity mask,
      gram G = Xblk @ X.T in fp8 DoubleRow (PE, 0.5 cyc/col),
      sim = G * invwT * invw_bc (DVE stt -> bf16),
      per-pair logit gather via gpsimd.local_scatter chains,
      exp/logsumexp (no slot mask needed: invalid pairs zeroed by pairmask),
      per-core partial sum -> [1,1].
  - host: sum 8 partials / P.
"""
import os
import sys
import numpy as np
import ml_dtypes

try:
    import concourse  # noqa: F401
except ImportError:
    sys.path.insert(0, "/opt/trn_rl_repo")

from contextlib import ExitStack

import concourse.bass as bass
import concourse.tile as tile
from concourse import bacc, mybir
from concourse._compat import with_exitstack
from concourse.bass_utils import run_bass_kernel_spmd

BF16 = ml_dtypes.bfloat16
FP8 = ml_dtypes.float8_e4m3
F32 = mybir.dt.float32
DBF = mybir.dt.bfloat16
F8 = mybir.dt.float8e4
I16 = mybir.dt.int16

B, D, J = 4096, 2048, 11
NCORES, RPC, NT, NKT, NKP = 8, 512, 4, 16, 8
TEMP = 0.1
AF = mybir.ActivationFunctionType
ALU = mybir.AluOpType
AX = mybir.AxisListType
DR = mybir.MatmulPerfMode.DoubleRow


# ---------------------------------------------------------------- host prep
def build_plan(anchor_idx, pos_idx, neg_idx):
    """Scatter planes; plane0 column indices are per-core ROTATED by -512k."""
    r = anchor_idx.astype(np.int64)
    cols = np.concatenate([pos_idx[:, None], neg_idx], axis=1).astype(np.int64)
    P = r.shape[0]

    order = np.argsort(r, kind="stable")
    r_sorted = r[order]
    first = np.r_[True, r_sorted[1:] != r_sorted[:-1]]
    gid = np.cumsum(first) - 1
    rank_sorted = np.arange(P) - np.flatnonzero(first)[gid]
    srank = np.empty(P, np.int64)
    srank[order] = rank_sorted
    n_per_row = np.bincount(r, minlength=B)
    SP = int(max(n_per_row.max(), 1))
    NE = SP * J + (SP * J) % 2
    assert NE * 32 < 2**16

    er = np.repeat(r, J)
    ec = cols.ravel()
    eslot = np.repeat(srank, J) * J + np.tile(np.arange(J), P)
    key = er * B + ec
    o2 = np.argsort(key, kind="stable")
    k_sorted = key[o2]
    first2 = np.r_[True, k_sorted[1:] != k_sorted[:-1]]
    gid2 = np.cumsum(first2) - 1
    occ_sorted = np.arange(P * J) - np.flatnonzero(first2)[gid2]
    occ = np.empty(P * J, np.int64)
    occ[o2] = occ_sorted
    L = int(occ.max())

    eslot_sorted = eslot[o2]
    prev_slot_sorted = np.empty(P * J, np.int64)
    prev_slot_sorted[0] = -1
    prev_slot_sorted[1:] = eslot_sorted[:-1]
    prev_slot = np.empty(P * J, np.int64)
    prev_slot[o2] = prev_slot_sorted

    core = er // RPC
    t = (er % RPC) // 128
    pp = er % 128
    ec_rot = (ec - core * RPC) % B  # per-core rotated column index

    plane0 = np.full((NCORES, NT, 128, B), -1, np.int16)
    m0 = occ == 0
    plane0[core[m0], t[m0], pp[m0], ec_rot[m0]] = eslot[m0].astype(np.int16)

    planes = []
    for q in range(1, L + 1):
        pl = np.full((NCORES, NT, 128, NE), -1, np.int16)
        mq = occ == q
        pl[core[mq], t[mq], pp[mq], prev_slot[mq]] = eslot[mq].astype(np.int16)
        planes.append(pl)

    nmat = n_per_row.reshape(NCORES, NT, 128)
    pairmask = (np.arange(SP)[None, None, None, :] < nmat[..., None]).astype(BF16)
    return dict(plane0=plane0, planes=planes, pairmask=pairmask, SP=SP, NE=NE, L=L)


# ------------------------------------------------------------- device kernel
@with_exitstack
def _build(ctx: ExitStack, tc: "tile.TileContext", io: dict, SP: int, NE: int, L: int):
    nc = tc.nc
    y_d, ident_d, pl0_d, pm_d, out_d = (
        io["y8"], io["ident"], io["plane0"], io["pm"], io["out"])
    plq_d = [io[f"plane{q}"] for q in range(1, L + 1)]
    NB = B // 128           # 32 column blocks
    BPB = 8                 # blocks per psum batch

    consts = ctx.enter_context(tc.tile_pool(name="consts", bufs=1))
    ones_f32c = consts.tile([128, 1], F32, tag="ones_f32c")
    nc.vector.memset(ones_f32c[:], 1.0)
    ident = consts.tile([128, 128], DBF, tag="ident")
    nc.sync.dma_start(ident[:], ident_d[:])

    ypool = ctx.enter_context(tc.tile_pool(name="y", bufs=1))
    y = ypool.tile([128, NKT, B], F8, tag="y", name="y")

    npool = ctx.enter_context(tc.tile_pool(name="norms", bufs=1))
    nrm2 = npool.tile([128, NB], F32, tag="nrm2")
    invw_col = npool.tile([128, NB], F32, tag="invw_col")
    invw_cb = npool.tile([128, NB], DBF, tag="invw_cb")
    invw_row = npool.tile([1, B], DBF, tag="invw_row")
    invw_bc = npool.tile([128, B], DBF, tag="invw_bc")

    lpool = ctx.enter_context(tc.tile_pool(name="loss", bufs=1))
    denall = lpool.tile([128, NT * SP], F32, tag="denall")
    l0all = lpool.tile([128, NT * SP], DBF, tag="l0all")
    pmall = lpool.tile([128, NT, SP], DBF, tag="pmall")

    # ---- x load (SP queue); pm + first planes also SP
    for kp in range(NKP):
        nc.sync.dma_start(y[:, 2 * kp:2 * kp + 2, :], y_d[:, 2 * kp:2 * kp + 2, :])

    # ---- norms from 32 block-diagonal self-grams (DoubleRow, no squares)
    with tc.tile_pool(name="bdg", bufs=2, space="PSUM") as bdg:
        bt = {}
        for bat in range(2):
            bt[bat] = bdg.tile([128, BPB * 128], F32, tag="bdg", name=f"bdg{bat}")
        for kp in range(NKP):
            for bat in range(2):
                for b in range(BPB):
                    blk = bat * BPB + b
                    nc.tensor.matmul(
                        bt[bat][:, b * 128:(b + 1) * 128],
                        lhsT=y[:, 2 * kp:2 * kp + 2, blk * 128:(blk + 1) * 128],
                        rhs=y[:, 2 * kp:2 * kp + 2, blk * 128:(blk + 1) * 128],
                        start=(kp == 0), stop=(kp == NKP - 1),
                        perf_mode=DR,
                    )
        scrapd = npool.tile([128, 128], DBF, tag="scrapd")
        for bat in range(4):
            if bat >= 2:
                bt[bat] = bdg.tile([128, BPB * 128], F32, tag="bdg", name=f"bdg{bat}")
                for kp in range(NKP):
                    for b in range(BPB):
                        blk = bat * BPB + b
                        nc.tensor.matmul(
                            bt[bat][:, b * 128:(b + 1) * 128],
                            lhsT=y[:, 2 * kp:2 * kp + 2, blk * 128:(blk + 1) * 128],
                            rhs=y[:, 2 * kp:2 * kp + 2, blk * 128:(blk + 1) * 128],
                            start=(kp == 0), stop=(kp == NKP - 1),
                            perf_mode=DR,
                        )
            for b in range(BPB):
                blk = bat * BPB + b
                nc.vector.scalar_tensor_tensor(
                    scrapd[:], bt[bat][:, b * 128:(b + 1) * 128], 1.0, ident[:],
                    ALU.mult, ALU.mult, accum_out=nrm2[:, blk:blk + 1])

        # invw_col = sqrt((1/TEMP)/nrm2); bf16 copy; transpose -> row layout
        nc.vector.reciprocal(invw_col[:], nrm2[:])
        nc.scalar.activation(invw_col[:], invw_col[:], AF.Sqrt, scale=1.0 / TEMP)
        nc.vector.tensor_copy(invw_cb[:], invw_col[:])
        ps_t = bdg.tile([NB, 128], DBF, tag="ps_t")
        nc.tensor.transpose(ps_t[:], invw_cb[:], ident[:])
        row_st = npool.tile([NB, 128], DBF, tag="row_st")
        nc.scalar.copy(row_st[:], ps_t[:])
        nc.sync.dma_start(invw_row[:], row_st[:])

    # pm for all tiles + plane prefetch for mt0 (SP queue)
    for t in range(NT):
        nc.sync.dma_start(pmall[:, t, :], pm_d[t])

    nc.gpsimd.partition_broadcast(invw_bc[:, 0:2048], invw_row[0:1, 0:2048])
    nc.gpsimd.partition_broadcast(invw_bc[:, 2048:B], invw_row[0:1, 2048:B])

    with tc.tile_pool(name="gpsum", bufs=2, space="PSUM") as gpsum, \
         tc.tile_pool(name="gbf", bufs=2) as gbfpool, \
         tc.tile_pool(name="pl0", bufs=2) as pl0pool, \
         tc.tile_pool(name="plq", bufs=2) as plqpool, \
         tc.tile_pool(name="slots", bufs=2) as slpool, \
         tc.tile_pool(name="elb", bufs=2) as elpool:

        # prefetch mt0 planes (SP queue)
        pl0_t = {0: pl0pool.tile([128, B], I16, tag="pl0", name="pl0_0")}
        nc.sync.dma_start(pl0_t[0][:], pl0_d[0])
        plq_t = {(0, q): plqpool.tile([128, NE], I16, tag=f"plq{q}", name=f"plq_0_{q}")
                 for q in range(1, L + 1)}
        for q in range(1, L + 1):
            nc.sync.dma_start(plq_t[(0, q)][:], plq_d[q - 1][0])

        # ---- per row-tile: gram (DoubleRow), normalize, scatter, partial loss
        for mt in range(NT):
            gbf = gbfpool.tile([128, B], DBF, tag="gbf")
            for half in range(2):
                gps = gpsum.tile([128, 2048], F32, tag="gram")
                for kp in range(NKP):
                    for chk in range(4):
                        c0 = half * 2048 + chk * 512
                        nc.tensor.matmul(
                            gps[:, chk * 512:(chk + 1) * 512],
                            lhsT=y[:, 2 * kp:2 * kp + 2, mt * 128:(mt + 1) * 128],
                            rhs=y[:, 2 * kp:2 * kp + 2, c0:c0 + 512],
                            start=(kp == 0), stop=(kp == NKP - 1),
                            perf_mode=DR,
                        )
                nc.vector.scalar_tensor_tensor(
                    gbf[:, half * 2048:(half + 1) * 2048], gps[:],
                    invw_col[:, mt:mt + 1],
                    invw_bc[:, half * 2048:(half + 1) * 2048],
                    ALU.mult, ALU.mult,
                )

            # prefetch next tile's planes (SP queue)
            if mt + 1 < NT:
                pl0_t[mt + 1] = pl0pool.tile([128, B], I16, tag="pl0", name=f"pl0_{mt+1}")
                nc.sync.dma_start(pl0_t[mt + 1][:], pl0_d[mt + 1])
                for q in range(1, L + 1):
                    plq_t[(mt + 1, q)] = plqpool.tile([128, NE], I16, tag=f"plq{q}", name=f"plq_{mt+1}_{q}")
                    nc.sync.dma_start(plq_t[(mt + 1, q)][:], plq_d[q - 1][mt + 1])

            # scatter chain
            s_lv = slpool.tile([128, NE], DBF, tag="slv0")
            nc.gpsimd.local_scatter(s_lv[:], gbf[:], pl0_t[mt][:], 128, NE, B)
            s_all = slpool.tile([128, NE], DBF, tag="s_all")
            nc.vector.tensor_copy(s_all[:], s_lv[:])
            for q in range(1, L + 1):
                s_nx = slpool.tile([128, NE], DBF, tag=f"slv{q % 2 + 1}")
                nc.gpsimd.local_scatter(s_nx[:], s_lv[:], plq_t[(mt, q)][:],
                                        128, NE, NE)
                nc.vector.tensor_tensor(s_all[:], s_all[:], s_nx[:], ALU.add)
                s_lv = s_nx

            # exp + per-pair denominators; Ln batched after the loop
            ebuf = elpool.tile([128, NE], F32, tag="ebuf")
            nc.scalar.activation(ebuf[:], s_all[:], AF.Exp)
            e3 = ebuf[:, 0:SP * J].rearrange("p (s j) -> p s j", j=J)
            nc.vector.tensor_reduce(denall[:, mt * SP:(mt + 1) * SP], e3,
                                    AX.X, ALU.add)
            l0 = s_all[:, 0:SP * J].rearrange("p (s j) -> p s j", j=J)[:, :, 0]
            nc.vector.tensor_copy(l0all[:, mt * SP:(mt + 1) * SP], l0)

    # ---- batched logsumexp tail + total
    with tc.tile_pool(name="p5psum", bufs=1, space="PSUM") as p5psum:
        lnd = lpool.tile([128, NT * SP], F32, tag="lnd")
        nc.scalar.activation(lnd[:], denall[:], AF.Ln)
        diff = lpool.tile([128, NT * SP], F32, tag="diff")
        nc.vector.scalar_tensor_tensor(diff[:], l0all[:], -1.0, lnd[:],
                                       ALU.mult, ALU.add)
        scrap = lpool.tile([128, NT * SP], F32, tag="scrap")
        acc1 = lpool.tile([128, 1], F32, tag="acc1")
        nc.vector.scalar_tensor_tensor(
            scrap[:], diff[:], 1.0,
            pmall[:].rearrange("p t s -> p (t s)"), ALU.mult, ALU.mult,
            accum_out=acc1[:],
        )
        ps = p5psum.tile([1, 1], F32, tag="ps_out")
        nc.tensor.matmul(ps[:], lhsT=acc1[:], rhs=ones_f32c[:, 0:1],
                         start=True, stop=True)
        res = lpool.tile([1, 1], F32, tag="res")
        nc.scalar.copy(res[:], ps[:])
        nc.sync.dma_start(out_d[:], res[:])


def build_nc(SP, NE, L, enable_asserts=False):
    nc = bacc.Bacc("TRN2", target_bir_lowering=False, debug=False,
                   enable_asserts=enable_asserts, num_devices=NCORES)
    io = {
        "y8": nc.dram_tensor("y8", [128, NKT, B], F8, kind="ExternalInput").ap(),
        "ident": nc.dram_tensor("ident", [128, 128], DBF, kind="ExternalInput").ap(),
        "plane0": nc.dram_tensor("plane0", [NT, 128, B], I16, kind="ExternalInput").ap(),
        "pm": nc.dram_tensor("pm", [NT, 128, SP], DBF, kind="ExternalInput").ap(),
        "out": nc.dram_tensor("out", [1, 1], F32, kind="ExternalOutput").ap(),
    }
    for q in range(1, L + 1):
        io[f"plane{q}"] = nc.dram_tensor(
            f"plane{q}", [NT, 128, NE], I16, kind="ExternalInput").ap()
    with tile.TileContext(nc) as tc:
        _build(tc, io, SP, NE, L)
    nc.compile()
    return nc


def make_in_maps(x, plan):
    x8 = np.clip(np.asarray(x, np.float32), -240.0, 240.0).astype(FP8)
    ident = np.eye(128, dtype=BF16)
    in_maps = []
    for k in range(NCORES):
        xr = np.roll(x8, -RPC * k, axis=0)          # [B, D]
        y8 = np.ascontiguousarray(
            xr.T.reshape(NKT, 128, B).transpose(1, 0, 2))  # [128, NKT, B]
        m = {
            "y8": y8,
            "ident": ident,
            "plane0": plan["plane0"][k],
            "pm": plan["pairmask"][k],
        }
        for q in range(1, plan["L"] + 1):
            m[f"plane{q}"] = plan["planes"][q - 1][k]
        in_maps.append(m)
    return in_maps


def kernel(**inputs):
    x = np.asarray(inputs["x"], np.float32)
    anchor_idx = np.asarray(inputs["anchor_idx"])
    pos_idx = np.asarray(inputs["pos_idx"])
    neg_idx = np.asarray(inputs["neg_idx"])
    P = anchor_idx.shape[0]

    plan = build_plan(anchor_idx, pos_idx, neg_idx)
    nc = build_nc(plan["SP"], plan["NE"], plan["L"])
    in_maps = make_in_maps(x, plan)
    res = run_bass_kernel_spmd(nc, in_maps, list(range(NCORES)))
    total = sum(float(res.results[k]["out"][0, 0]) for k in range(NCORES))
    return np.float32(total / P)
